# revision 24
# baseline (speedup 1.0000x reference)
"""Trainium2 Bass kernel for nn_Attention_88467736363354.

Reference computation (B=256, S=1024, C=256 in-features, H=8 heads, D=64):
    q = einsum('bi,hdi->bhd', queries, Wq) + bq            # (B, H, D)
    k = einsum('bsi,hdi->bhsd', keys, Wk) + bk             # (B, H, S, D)
    scores = einsum('bhd,bhsd->bhs', q, k) / sqrt(D)
    out = softmax(scores) @ values                          # (B, H, 1, V)

Algebraic rewrite used here (exact up to fp reassociation):
    scores[b,h,s] = keys[b,s,:] . qk[b,h,:] + (q[b,h,:] . bk[h]) / 8
    with qk[b,h,:] = q[b,h,:] @ (Wk[h] / 8)
The q.bk term is constant over s, so it cancels in softmax and is dropped.
This avoids projecting all keys (30x less FLOPs); the kernel is then
memory-bound: 512 MB of keys+values streamed across 8 NeuronCores.

Sharding: data-parallel over batch. Each of the 8 cores handles 32 batch
elements; the small projection weights are replicated. keys are passed
per-batch transposed (contraction dim on partitions) — a host-side layout
choice during sharding.
"""
import sys

for _p in ("/opt/trn_rl_repo",):
    if _p not in sys.path:
        sys.path.insert(0, _p)

import numpy as np

B, S, C, H, D, V = 256, 1024, 256, 8, 64, 256
NCORES = 8
BL = B // NCORES            # 32 batch elements per core
NQ = BL // 4                # 8 quads of 4

# const tile column offsets
_WQ, _QT, _BQ, _WK, _ID = 0, 1024, 1088, 1092, 3140
_CONSTF = 3268

_cache = {}


def _build_nc():
    import concourse.tile as tile
    from concourse import bacc, mybir
    from contextlib import ExitStack

    F32 = mybir.dt.float32
    BF16 = mybir.dt.bfloat16
    EXP = mybir.ActivationFunctionType.Exp
    IDENT = mybir.ActivationFunctionType.Identity

    nc = bacc.Bacc(num_swdge_queues=4)
    const_ext = nc.declare_dram_parameter("const0", [128, _CONSTF], F32, isOutput=False)
    keys_ext = nc.declare_dram_parameter("keysT", [BL, C, S], F32, isOutput=False)
    vals_ext = nc.declare_dram_parameter("vals", [BL, S, V], F32, isOutput=False)
    out_ext = nc.declare_dram_parameter("out", [BL * H, V], F32, isOutput=True)
    identb_ext = nc.declare_dram_parameter("identb", [128, 128], BF16, isOutput=False)

    with tile.TileContext(nc) as tc, ExitStack() as ctx:
        singles = ctx.enter_context(tc.tile_pool(name="singles", bufs=1))
        const = singles.tile([128, _CONSTF], F32)
        nc.sync.dma_start(const[:], const_ext[:])
        wq = const[:, _WQ:_WQ + 1024]
        qT = const[:, _QT:_QT + 64]
        bqp = const[:, _BQ:_BQ + 4]
        wk = const[:, _WK:_WK + 2048]
        ident = const[:, _ID:_ID + 128]
        identb = singles.tile([128, 128], BF16)
        nc.sync.dma_start(identb[:], identb_ext[:])

        # DVE pre-touch of const: absorbs the const-DMA wait for DVE
        scratch = singles.tile([128, 4], F32)
        nc.vector.tensor_copy(scratch[:], bqp[:])

        q_sb = singles.tile([128, 128], F32)
        qkT = singles.tile([128, 512], F32)
        qkTb = singles.tile([128, 512], BF16)

        # main-loop pools (PSUM budget: 2 + 2 + 4 = 8 banks)
        kt_pool = ctx.enter_context(tc.tile_pool(name="kt", bufs=12))
        vt_pool = ctx.enter_context(tc.tile_pool(name="vt", bufs=16))
        attn2_pool = ctx.enter_context(tc.tile_pool(name="attn2", bufs=2))
        attnT_pool = ctx.enter_context(tc.tile_pool(name="attnT", bufs=2))
        rec_pool = ctx.enter_context(tc.tile_pool(name="rec", bufs=2))
        osb_pool = ctx.enter_context(tc.tile_pool(name="osb", bufs=BL))
        ps4_pool = ctx.enter_context(tc.tile_pool(name="ps4", bufs=1, space="PSUM"))
        pstr_pool = ctx.enter_context(tc.tile_pool(name="pstr", bufs=2, space="PSUM"))
        psav_pool = ctx.enter_context(tc.tile_pool(name="psav", bufs=4, space="PSUM"))

        # ---- phase 0: project q, build qkT ----
        # phase-0 psum tiles borrow the psav pool's slots (tag-shared)
        # q_all[hd, b] = Wq_flat @ queries_shard.T  (chunked over hd and i)
        psq = psav_pool.tile([128, 128], F32, tag="psav")
        for c in range(4):
            for ic in range(2):
                nc.tensor.matmul(
                    psq[:, 32 * c:32 * c + 32],
                    wq[:, 512 * ic + 128 * c:512 * ic + 128 * c + 128],
                    qT[:, 32 * ic:32 * ic + 32],
                    start=(ic == 0), stop=(ic == 1))
        for c in range(4):
            nc.vector.tensor_scalar_add(
                q_sb[:, 32 * c:32 * c + 32], psq[:, 32 * c:32 * c + 32],
                bqp[:, c:c + 1])
        # qkT[i, 32h + b] = sum_d q[b,h,d] * Wk_s[h,d,i], chunked over i.
        # wk is zero-padded to K=128 per head (host side) so all matmuls use
        # row position 0 — mixed row tile positions sharing a PSUM bank run
        # concurrently on different PE sub-arrays and hard-fault the device.
        psqk = psav_pool.tile([128, 512], F32, tag="psav")
        for h in range(H):
            c = h // 2
            for ic in range(2):
                nc.tensor.matmul(
                    psqk[:, 256 * ic + 32 * h:256 * ic + 32 * h + 32],
                    wk[:, 256 * h + 128 * ic:256 * h + 128 * ic + 128],
                    q_sb[:, 32 * c:32 * c + 32],
                    start=True, stop=True)
        nc.vector.tensor_copy(qkT[:, 0:256], psqk[:, 0:256])
        nc.vector.tensor_copy(qkT[:, 256:512], psqk[:, 256:512])
        nc.vector.tensor_copy(qkTb[:], qkT[:])
        # PE fence: observe the DVE tick (qkT copies/memset) before the main loop
        ps_fence = pstr_pool.tile([128, 128], BF16, tag="pstr")
        nc.tensor.transpose(ps_fence[:], qkTb[:, 0:128], identb[:])

        # persistent parity tiles for attn/sume: dead rows (24 per 32-row
        # band) are zeroed once here and never touched again, so the
        # full-tile scale/transpose always reads defined data
        attn_par = [singles.tile([128, 1024], F32, name=f"attn{i}") for i in range(2)]
        sume_par = [singles.tile([128, 1], F32, name=f"sume{i}") for i in range(2)]
        for i in range(2):
            nc.vector.memset(attn_par[i][:], 0.0)
            nc.vector.memset(sume_par[i][:], 1.0)

        # ---- main loop: 8 quads of 4 batch elements ----
        import os
        nq_run = int(os.environ.get("KERNEL_NQ", NQ))

        def issue_keys_quad(G):
            tiles = []
            for r in range(4):
                b = 4 * G + r
                kt = kt_pool.tile([128, 2048], BF16, name=f"kt{b}", tag="kt")
                nc.gpsimd.dma_start(
                    kt[:].rearrange("p (c s) -> p c s", c=2),
                    keys_ext[b].rearrange("(c p) s -> p c s", c=2))
                tiles.append(kt)
            return tiles

        # keys stay 2 quads ahead of vals in the SWDGE queue so the last
        # quad's QK/softmax/transposes overlap the final vals streaming
        kq = {}
        for G in range(min(2, nq_run)):
            kq[G] = issue_keys_quad(G)

        for G in range(nq_run):
            attn = attn_par[G % 2]
            sume = sume_par[G % 2]
            attn2 = attn2_pool.tile([128, 1024], BF16)
            attnT = attnT_pool.tile([128, 1024], BF16)
            rec = rec_pool.tile([128, 1], F32)
            ps4 = ps4_pool.tile([128, 1024], F32)

            for r in range(4):
                b = 4 * G + r
                kt = kq[G][r]
                for sh in range(2):
                    for kc in range(2):
                        nc.tensor.matmul(
                            ps4[32 * r:32 * r + 8, 512 * sh:512 * sh + 512],
                            qkTb[:, 256 * kc + b:256 * kc + b + 7 * 32 + 1:32],
                            kt[:, 1024 * kc + 512 * sh:1024 * kc + 512 * sh + 512],
                            start=(kc == 0), stop=(kc == 1),
                            tile_position=(0, 32 * r))
            del kq[G]

            for r in range(4):
                nc.scalar.activation(attn[32 * r:32 * r + 8, :],
                                     ps4[32 * r:32 * r + 8, :], EXP,
                                     accum_out=sume[32 * r:32 * r + 8, :])
            nc.vector.reciprocal(rec[:], sume[:])
            nc.scalar.activation(attn2[:], attn[:], IDENT, scale=rec[:])

            for t in range(8):
                pstr = pstr_pool.tile([128, 128], BF16)
                nc.tensor.transpose(pstr[:], attn2[:, 128 * t:128 * t + 128], identb[:])
                nc.vector.tensor_copy(attnT[:, 128 * t:128 * t + 128], pstr[:])

            if G + 2 < nq_run:
                kq[G + 2] = issue_keys_quad(G + 2)

            for r in range(4):
                b = 4 * G + r
                vt = vt_pool.tile([128, 2048], BF16)
                nc.gpsimd.dma_start(
                    vt[:].rearrange("p (t v) -> p t v", t=8),
                    vals_ext[b].rearrange("(t p) v -> p t v", t=8))
                psav = psav_pool.tile([8, 256], F32)
                for t in range(8):
                    nc.tensor.matmul(
                        psav[:, :],
                        attnT[:, 128 * t + 32 * r:128 * t + 32 * r + 8],
                        vt[:, 256 * t:256 * t + 256],
                        start=(t == 0), stop=(t == 7))
                osb = osb_pool.tile([8, 256], F32)
                nc.scalar.copy(osb[:], psav[:])
                nc.scalar.dma_start(out_ext[8 * b:8 * b + 8, :], osb[:])

    nc.compile()
    return nc


def _host_prep(queries, keys, values, Wq, bq, Wk, bk):
    """Build per-core input maps (sharding + layout marshalling)."""
    f32 = np.float32
    queries = np.asarray(queries, f32)
    keys = np.asarray(keys, f32)
    values = np.asarray(values, f32)
    Wq = np.asarray(Wq, f32)
    bq = np.asarray(bq, f32)
    Wk = np.asarray(Wk, f32)

    # replicated const block (core-independent parts)
    Wq_flat = Wq.reshape(H * D, C)                        # (512, 256)
    Wt = Wq_flat.T                                        # (256, 512)
    wq_host = np.concatenate([Wt[0:128, :], Wt[128:256, :]], axis=1)  # (128,1024)
    bq_host = bq.reshape(H * D).reshape(4, 128).T         # (128, 4)
    Wk_s = (Wk / 8.0).astype(f32)                         # exact: /8 is 2^-3
    # zero-padded K=128 layout: head h's 64 d-rows sit at partition range
    # [64*(h%2), 64*(h%2)+64); the other 64 rows are zero
    wk_host = np.zeros((128, H * C), f32)                 # (128, 2048)
    for h in range(H):
        j = h % 2
        wk_host[64 * j:64 * j + 64, C * h:C * (h + 1)] = Wk_s[h].reshape(D, C)
    id_host = np.eye(128, dtype=f32)

    keysT = np.swapaxes(keys, 1, 2)                       # (B, C, S) view

    import ml_dtypes
    identb = np.eye(128).astype(ml_dtypes.bfloat16)

    in_maps = []
    for m in range(NCORES):
        qs = queries[BL * m:BL * (m + 1), :].T            # (256, 32)
        qT_host = np.concatenate([qs[0:128, :], qs[128:256, :]], axis=1)  # (128,64)
        const0 = np.concatenate(
            [wq_host, qT_host, bq_host, wk_host, id_host], axis=1).astype(f32)
        assert const0.shape == (128, _CONSTF)
        in_maps.append({
            "const0": np.ascontiguousarray(const0),
            "keysT": np.ascontiguousarray(keysT[BL * m:BL * (m + 1)]),
            "vals": np.ascontiguousarray(values[BL * m:BL * (m + 1)]),
            "identb": identb,
        })
    return in_maps


def _run(inputs, trace=False, **trace_kwargs):
    from concourse.bass_utils import run_bass_kernel_spmd

    if "nc" not in _cache:
        _cache["nc"] = _build_nc()
    nc = _cache["nc"]
    in_maps = _host_prep(**inputs)
    res = run_bass_kernel_spmd(nc, in_maps, list(range(NCORES)),
                               trace=trace, **trace_kwargs)
    outs = [res.results[m]["out"].reshape(BL, H, 1, V) for m in range(NCORES)]
    full = np.concatenate(outs, axis=0)
    return full, res


def kernel(**inputs):
    full, _ = _run(inputs, trace=False)
    return full


# revision 32
# speedup vs baseline: 1.2185x; 1.2185x over previous
"""Trainium2 Bass kernel for nn_Attention_88467736363354.

Reference computation (B=256, S=1024, C=256 in-features, H=8 heads, D=64):
    q = einsum('bi,hdi->bhd', queries, Wq) + bq            # (B, H, D)
    k = einsum('bsi,hdi->bhsd', keys, Wk) + bk             # (B, H, S, D)
    scores = einsum('bhd,bhsd->bhs', q, k) / sqrt(D)
    out = softmax(scores) @ values                          # (B, H, 1, V)

Algebraic rewrite used here (exact up to fp reassociation):
    scores[b,h,s] = keys[b,s,:] . qk[b,h,:] + (q[b,h,:] . bk[h]) / 8
    with qk[b,h,:] = q[b,h,:] @ (Wk[h] / 8)
The q.bk term is constant over s, so it cancels in softmax and is dropped.
This avoids projecting all keys (30x less FLOPs); the kernel is then
memory-bound: 512 MB of keys+values streamed across 8 NeuronCores.

Sharding: data-parallel over batch. Each of the 8 cores handles 32 batch
elements; the small projection weights are replicated. keys are passed
per-batch transposed (contraction dim on partitions) — a host-side layout
choice during sharding.
"""
import sys

for _p in ("/opt/trn_rl_repo",):
    if _p not in sys.path:
        sys.path.insert(0, _p)

import numpy as np

B, S, C, H, D, V = 256, 1024, 256, 8, 64, 256
NCORES = 8
BL = B // NCORES            # 32 batch elements per core
NQ = BL // 4                # 8 quads of 4

# const tile column offsets
_WQ, _QT, _BQ, _WK, _ID = 0, 1024, 1088, 1092, 3140
_CONSTF = 3204

_cache = {}


def _build_nc():
    import concourse.tile as tile
    from concourse import bacc, mybir
    from contextlib import ExitStack

    F32 = mybir.dt.float32
    BF16 = mybir.dt.bfloat16
    EXP = mybir.ActivationFunctionType.Exp
    IDENT = mybir.ActivationFunctionType.Identity

    nc = bacc.Bacc(num_swdge_queues=4)
    const_ext = nc.declare_dram_parameter("const0", [128, _CONSTF], F32, isOutput=False)
    keys_ext = nc.declare_dram_parameter("keysT", [BL, C, S], F32, isOutput=False)
    vals_ext = nc.declare_dram_parameter("vals", [BL, S, V], F32, isOutput=False)
    out_ext = nc.declare_dram_parameter("out", [BL * H, V], F32, isOutput=True)

    with tile.TileContext(nc) as tc, ExitStack() as ctx:
        singles = ctx.enter_context(tc.tile_pool(name="singles", bufs=1))
        const = singles.tile([128, _CONSTF], F32)
        nc.sync.dma_start(const[:], const_ext[:])
        wq = const[:, _WQ:_WQ + 1024]
        qT = const[:, _QT:_QT + 64]
        bqp = const[:, _BQ:_BQ + 4]
        wk = const[:, _WK:_WK + 2048]
        identb = const[:, _ID:_ID + 64].bitcast(mybir.dt.bfloat16)

        # DVE pre-touch of const: absorbs the const-DMA wait for DVE
        scratch = singles.tile([128, 4], F32)
        nc.vector.tensor_copy(scratch[:], bqp[:])

        q_sb = singles.tile([128, 128], F32)
        qkT = singles.tile([128, 512], F32)
        qkTb = singles.tile([128, 512], BF16)

        # main-loop pools (PSUM budget: 2 + 2 + 4 = 8 banks)
        kt_pool = ctx.enter_context(tc.tile_pool(name="kt", bufs=12))
        attnT_pool = ctx.enter_context(tc.tile_pool(name="attnT", bufs=2))
        rec_pool = ctx.enter_context(tc.tile_pool(name="rec", bufs=4))
        osb_pool = ctx.enter_context(tc.tile_pool(name="osb", bufs=BL))
        ps4_pool = ctx.enter_context(tc.tile_pool(name="ps4", bufs=1, space="PSUM"))
        pstr_pool = ctx.enter_context(tc.tile_pool(name="pstr", bufs=2, space="PSUM"))
        psav_pool = ctx.enter_context(tc.tile_pool(name="psav", bufs=4, space="PSUM"))

        # ---- phase 0: project q, build qkT ----
        # phase-0 psum tiles borrow the psav pool's slots (tag-shared)
        # q_all[hd, b] = Wq_flat @ queries_shard.T  (chunked over hd and i)
        psq = psav_pool.tile([128, 128], F32, tag="psav")
        for c in range(4):
            for ic in range(2):
                nc.tensor.matmul(
                    psq[:, 32 * c:32 * c + 32],
                    wq[:, 512 * ic + 128 * c:512 * ic + 128 * c + 128],
                    qT[:, 32 * ic:32 * ic + 32],
                    start=(ic == 0), stop=(ic == 1))
        for c in range(4):
            nc.vector.tensor_scalar_add(
                q_sb[:, 32 * c:32 * c + 32], psq[:, 32 * c:32 * c + 32],
                bqp[:, c:c + 1])
        # qkT[i, 32h + b] = sum_d q[b,h,d] * Wk_s[h,d,i], chunked over i.
        # wk is zero-padded to K=128 per head (host side) so all matmuls use
        # row position 0 — mixed row tile positions sharing a PSUM bank run
        # concurrently on different PE sub-arrays and hard-fault the device.
        psqk = psav_pool.tile([128, 512], F32, tag="psav")
        for h in range(H):
            c = h // 2
            for ic in range(2):
                nc.tensor.matmul(
                    psqk[:, 256 * ic + 32 * h:256 * ic + 32 * h + 32],
                    wk[:, 256 * h + 128 * ic:256 * h + 128 * ic + 128],
                    q_sb[:, 32 * c:32 * c + 32],
                    start=True, stop=True)
        nc.vector.tensor_copy(qkT[:, 0:256], psqk[:, 0:256])
        nc.vector.tensor_copy(qkT[:, 256:512], psqk[:, 256:512])
        nc.vector.tensor_copy(qkTb[:], qkT[:])
        # PE fence: observe the DVE tick (qkT copies/memset) before the main loop
        ps_fence = pstr_pool.tile([128, 128], BF16, tag="pstr")
        nc.tensor.transpose(ps_fence[:], qkTb[:, 0:128], identb[:])

        # persistent parity tiles for attn: dead rows (24 per 32-row band)
        # are zeroed once here and never touched again, so the full-tile
        # transpose always reads defined data
        attn_par = [singles.tile([128, 1024], BF16, name=f"attn{i}") for i in range(2)]
        for i in range(2):
            nc.vector.memset(attn_par[i][:], 0.0)

        # vals ring tiles: 8 blocks of [256 vals cols + 1 ones col]; the ones
        # columns are written once and reused, turning the AV matmul's 257th
        # column into the softmax denominator
        vt_ring = [singles.tile([128, 2056], BF16, name=f"vt{i}") for i in range(16)]
        for i in range(16):
            nc.vector.memset(
                vt_ring[i][:].rearrange("p (t x) -> p t x", t=8)[:, :, 256:257], 1.0)

        # ---- main loop: 8 quads of 4 batch elements ----
        nq_run = NQ

        def issue_keys_quad(G):
            tiles = []
            for r in range(4):
                b = 4 * G + r
                kt = kt_pool.tile([128, 2048], BF16, name=f"kt{b}", tag="kt")
                nc.gpsimd.dma_start(
                    kt[:].rearrange("p (c s) -> p c s", c=2),
                    keys_ext[b].rearrange("(c p) s -> p c s", c=2))
                tiles.append(kt)
            return tiles

        # keys stay 2 quads ahead of vals in the SWDGE queue so the last
        # quad's QK/softmax/transposes overlap the final vals streaming
        kq = {}
        for G in range(min(2, nq_run)):
            kq[G] = issue_keys_quad(G)

        for G in range(nq_run):
            attn = attn_par[G % 2]
            attnT = attnT_pool.tile([128, 1024], BF16)
            ps4 = ps4_pool.tile([128, 1024], F32)

            for r in range(4):
                b = 4 * G + r
                kt = kq[G][r]
                for sh in range(2):
                    for kc in range(2):
                        nc.tensor.matmul(
                            ps4[32 * r:32 * r + 8, 512 * sh:512 * sh + 512],
                            qkTb[:, 256 * kc + b:256 * kc + b + 7 * 32 + 1:32],
                            kt[:, 1024 * kc + 512 * sh:1024 * kc + 512 * sh + 512],
                            start=(kc == 0), stop=(kc == 1),
                            tile_position=(0, 32 * r))
            del kq[G]

            for r in range(4):
                b = 4 * G + r
                vt = vt_ring[b % 16]
                nc.gpsimd.dma_start(
                    vt[:].rearrange("p (t x) -> p t x", t=8)[:, :, 0:256],
                    vals_ext[b].rearrange("(t p) v -> p t v", t=8))

            for r in range(4):
                nc.scalar.activation(attn[32 * r:32 * r + 8, :],
                                     ps4[32 * r:32 * r + 8, :], EXP)

            for t in range(8):
                pstr = pstr_pool.tile([128, 128], BF16)
                nc.tensor.transpose(pstr[:], attn[:, 128 * t:128 * t + 128], identb[:])
                nc.vector.tensor_copy(attnT[:, 128 * t:128 * t + 128], pstr[:])

            if G + 2 < nq_run:
                kq[G + 2] = issue_keys_quad(G + 2)

            for r in range(4):
                b = 4 * G + r
                vt = vt_ring[b % 16]
                psav = psav_pool.tile([8, 257], F32, tag="psav")
                for t in range(8):
                    nc.tensor.matmul(
                        psav[:, :],
                        attnT[:, 128 * t + 32 * r:128 * t + 32 * r + 8],
                        vt[:, 257 * t:257 * t + 257],
                        start=(t == 0), stop=(t == 7))
                rec_b = rec_pool.tile([8, 1], F32, tag="rec")
                nc.vector.reciprocal(rec_b[:], psav[0:8, 256:257])
                osb = osb_pool.tile([8, 256], F32)
                nc.vector.tensor_scalar_mul(osb[:], psav[0:8, 0:256], rec_b[:])
                nc.scalar.dma_start(out_ext[8 * b:8 * b + 8, :], osb[:])

    nc.compile()
    return nc


def _host_prep(queries, keys, values, Wq, bq, Wk, bk):
    """Build per-core input maps (sharding + layout marshalling)."""
    f32 = np.float32
    queries = np.asarray(queries, f32)
    keys = np.asarray(keys, f32)
    values = np.asarray(values, f32)
    Wq = np.asarray(Wq, f32)
    bq = np.asarray(bq, f32)
    Wk = np.asarray(Wk, f32)

    # replicated const block (core-independent parts)
    Wq_flat = Wq.reshape(H * D, C)                        # (512, 256)
    Wt = Wq_flat.T                                        # (256, 512)
    wq_host = np.concatenate([Wt[0:128, :], Wt[128:256, :]], axis=1)  # (128,1024)
    bq_host = bq.reshape(H * D).reshape(4, 128).T         # (128, 4)
    Wk_s = (Wk / 8.0).astype(f32)                         # exact: /8 is 2^-3
    # zero-padded K=128 layout: head h's 64 d-rows sit at partition range
    # [64*(h%2), 64*(h%2)+64); the other 64 rows are zero
    wk_host = np.zeros((128, H * C), f32)                 # (128, 2048)
    for h in range(H):
        j = h % 2
        wk_host[64 * j:64 * j + 64, C * h:C * (h + 1)] = Wk_s[h].reshape(D, C)

    keysT = np.swapaxes(keys, 1, 2)                       # (B, C, S) view

    import ml_dtypes
    id_host = np.ascontiguousarray(
        np.eye(128).astype(ml_dtypes.bfloat16)).view(np.float32)  # (128, 64)

    in_maps = []
    for m in range(NCORES):
        qs = queries[BL * m:BL * (m + 1), :].T            # (256, 32)
        qT_host = np.concatenate([qs[0:128, :], qs[128:256, :]], axis=1)  # (128,64)
        const0 = np.concatenate(
            [wq_host, qT_host, bq_host, wk_host, id_host], axis=1).astype(f32)
        assert const0.shape == (128, _CONSTF)
        in_maps.append({
            "const0": np.ascontiguousarray(const0),
            "keysT": np.ascontiguousarray(keysT[BL * m:BL * (m + 1)]),
            "vals": np.ascontiguousarray(values[BL * m:BL * (m + 1)]),
        })
    return in_maps


def _run(inputs, trace=False, **trace_kwargs):
    from concourse.bass_utils import run_bass_kernel_spmd

    if "nc" not in _cache:
        _cache["nc"] = _build_nc()
    nc = _cache["nc"]
    in_maps = _host_prep(**inputs)
    res = run_bass_kernel_spmd(nc, in_maps, list(range(NCORES)),
                               trace=trace, **trace_kwargs)
    outs = [res.results[m]["out"].reshape(BL, H, 1, V) for m in range(NCORES)]
    full = np.concatenate(outs, axis=0)
    return full, res


def kernel(**inputs):
    full, _ = _run(inputs, trace=False)
    return full
